# revision 1
# baseline (speedup 1.0000x reference)
"""RNN-T Joiner kernel for Trainium2 (Bass/Tile), 8-core data-parallel over batch.

out[b,t,u,v] = (enc[b,t] @ We)[v] + (pred[b,u] @ Wp)[v] + bias[v]

Per core (one batch element):
  - PE (fp32): enc_proj [256,1024] and pred_b [65,1024] projections.
  - PE (fp32r): broadcast pred_b rows across the 128 t-partitions via one-hot
    selection matmuls into PSUM. Even u rows live at partitions 0-32, odd u
    rows at partitions 64-95, so consecutive matmuls alternate PE row groups
    and LDWEIGHTS overlaps in-flight MATMULs (64-deep reorder window).
  - DVE: one tensor_tensor add per output element (the mandatory PSUM->SBUF
    trip) producing staged output tiles.
  - HWDGE DMA: 10 uniform 6.8 MB contiguous stores (13 u's per block).
"""

import sys

sys.path.insert(0, "/opt/trn_rl_repo")

import numpy as np

B, T, U1, D, V = 8, 256, 65, 640, 1024
KC = D // 128  # 5 contraction chunks
UBLK = 13      # u's per output DMA block: 5 blocks x 13 = 65
NBLK = U1 // UBLK
NE = (U1 + 1) // 2  # 33 even u rows (0,2,..,64)
NO = U1 // 2        # 32 odd u rows (1,3,..,63)

_COMPILED = None


def _build():
    import concourse.bacc as bacc
    import concourse.tile as tile
    import concourse.mybir as mybir

    f32 = mybir.dt.float32
    f32r = mybir.dt.float32r

    nc = bacc.Bacc("TRN2", target_bir_lowering=False, debug=False, num_devices=8)

    encT = nc.dram_tensor("encT", [D, T], f32, kind="ExternalInput")
    # predT columns: even u's (0,2,..,64) then odd u's (1,3,..,63)
    predT = nc.dram_tensor("predT", [D, U1], f32, kind="ExternalInput")
    W = nc.dram_tensor("W", [2 * D, V], f32, kind="ExternalInput")
    bias = nc.dram_tensor("bias", [1, V], f32, kind="ExternalInput")
    ones = nc.dram_tensor("ones", [1, 128], f32, kind="ExternalInput")
    # packed one-hot: rows 0-32 select even u (identity33 x ones128),
    # rows 64-95 select odd u (identity32 x ones128)
    sel = nc.dram_tensor("sel", [128, NE * 128], f32r, kind="ExternalInput")
    out = nc.dram_tensor("out", [T, U1 * V], f32, kind="ExternalOutput")

    with tile.TileContext(nc) as tc:
        with tc.tile_pool(name="consts", bufs=1) as cp:
            sel_sb = cp.tile([128, NE * 128], f32r, tag="sel")
            pred_sp = cp.tile([128, V], f32r, tag="pred_sp")
            enc_dup = []
            for tt in range(2):
                t_ = cp.tile([128, 2 * V], f32, tag=f"enc_dup{tt}")
                enc_dup.append(t_)

            with tc.tile_pool(name="wpool", bufs=1) as wp:
                predT_sb = []
                Wp_sb = []
                encT_sb = []
                We_sb = []
                for c in range(KC):
                    t_ = wp.tile([128, U1], f32, tag=f"predT{c}")
                    nc.sync.dma_start(t_[:], predT[c * 128:(c + 1) * 128, :])
                    predT_sb.append(t_)
                    t_ = wp.tile([128, V], f32, tag=f"Wp{c}")
                    nc.sync.dma_start(t_[:], W[D + c * 128:D + (c + 1) * 128, :])
                    Wp_sb.append(t_)
                bias_sb = wp.tile([1, V], f32, tag="bias")
                nc.sync.dma_start(bias_sb[:], bias[:])
                ones_sb = wp.tile([1, 128], f32, tag="ones")
                nc.sync.dma_start(ones_sb[:], ones[:])
                for c in range(KC):
                    t_ = wp.tile([128, T], f32, tag=f"encT{c}")
                    nc.sync.dma_start(t_[:], encT[c * 128:(c + 1) * 128, :])
                    encT_sb.append(t_)
                    t_ = wp.tile([128, V], f32, tag=f"We{c}")
                    nc.sync.dma_start(t_[:], W[c * 128:(c + 1) * 128, :])
                    We_sb.append(t_)
                nc.sync.dma_start(sel_sb[:], sel[:])

                # ---- setup: projections (fp32 PE matmuls) ----
                with tc.tile_pool(name="spsum", bufs=2, space="PSUM") as sp:
                    ps_p = sp.tile([128, V], f32, tag="ps")
                    for vt in range(2):
                        vs = slice(vt * 512, (vt + 1) * 512)
                        for c in range(KC):
                            nc.tensor.matmul(
                                ps_p[0:NE, vs], predT_sb[c][:, 0:NE],
                                Wp_sb[c][:, vs], start=(c == 0), stop=False)
                        nc.tensor.matmul(
                            ps_p[0:NE, vs], ones_sb[0:1, 0:NE], bias_sb[0:1, vs],
                            start=False, stop=True)
                    for vt in range(2):
                        vs = slice(vt * 512, (vt + 1) * 512)
                        for c in range(KC):
                            nc.tensor.matmul(
                                ps_p[64:64 + NO, vs], predT_sb[c][:, NE:U1],
                                Wp_sb[c][:, vs], start=(c == 0), stop=False)
                        nc.tensor.matmul(
                            ps_p[64:64 + NO, vs], ones_sb[0:1, 0:NO], bias_sb[0:1, vs],
                            start=False, stop=True)
                    nc.vector.tensor_copy(pred_sp[0:NE, :], ps_p[0:NE, :])
                    nc.vector.tensor_copy(pred_sp[64:64 + NO, :], ps_p[64:64 + NO, :])

                    for tt in range(2):
                        ts_ = slice(tt * 128, (tt + 1) * 128)
                        ps_e = sp.tile([128, V], f32, tag="pse")
                        for vt in range(2):
                            vs = slice(vt * 512, (vt + 1) * 512)
                            for c in range(KC):
                                nc.tensor.matmul(
                                    ps_e[:, vs], encT_sb[c][:, ts_], We_sb[c][:, vs],
                                    start=(c == 0), stop=(c == KC - 1))
                        nc.vector.tensor_copy(enc_dup[tt][:, 0:V], ps_e[:])
                        nc.vector.tensor_copy(enc_dup[tt][:, V:2 * V], ps_e[:])

            def bcast_mm(ps_ap, u, vt):
                # one [128,512] slice of pred_b[u] broadcast to all partitions
                vs = slice(vt * 512, (vt + 1) * 512)
                if u % 2 == 0:
                    nc.tensor.matmul(
                        ps_ap, sel_sb[0:NE, (u // 2) * 128:(u // 2 + 1) * 128],
                        pred_sp[0:NE, vs], start=True, stop=True)
                else:
                    nc.tensor.matmul(
                        ps_ap, sel_sb[64:64 + NO, (u // 2) * 128:(u // 2 + 1) * 128],
                        pred_sp[64:64 + NO, vs], start=True, stop=True)

            # ---- main loop: broadcast-add-store ----
            # psum broadcast tiles are identical for both t-halves: compute
            # once, add into both t-stages (halves PE work).
            with tc.tile_pool(name="outp", bufs=2) as op_, \
                 tc.tile_pool(name="mpsum", bufs=2, space="PSUM") as mp:
                for blk in range(9):
                    u0 = blk * 8
                    nu = 8 if blk < 7 else 4
                    if blk == 8:
                        u0 = 60
                    stage0 = op_.tile([128, 8 * V], f32, tag="stage0")
                    stage1 = op_.tile([128, 8 * V], f32, tag="stage1")
                    for pair in range(nu // 2):
                        ua = u0 + 2 * pair
                        ps = mp.tile([128, 2048], f32, tag="mps")
                        bcast_mm(ps[:, 0:512], ua, 0)
                        bcast_mm(ps[:, 1024:1536], ua + 1, 0)
                        bcast_mm(ps[:, 512:1024], ua, 1)
                        bcast_mm(ps[:, 1536:2048], ua + 1, 1)
                        nc.vector.tensor_add(
                            stage0[:, pair * 2048:(pair + 1) * 2048],
                            enc_dup[0][:], ps[:])
                        nc.vector.tensor_add(
                            stage1[:, pair * 2048:(pair + 1) * 2048],
                            enc_dup[1][:], ps[:])
                    nc.sync.dma_start(
                        out[0:128, u0 * V:(u0 + nu) * V], stage0[:, 0:nu * V])
                    nc.sync.dma_start(
                        out[128:256, u0 * V:(u0 + nu) * V], stage1[:, 0:nu * V])
                # tail u = 64
                u = U1 - 1
                stage0 = op_.tile([128, 8 * V], f32, tag="stage0")
                stage1 = op_.tile([128, 8 * V], f32, tag="stage1")
                ps = mp.tile([128, 2048], f32, tag="mps")
                bcast_mm(ps[:, 0:512], u, 0)
                bcast_mm(ps[:, 512:1024], u, 1)
                nc.vector.tensor_add(stage0[:, 0:V], enc_dup[0][:, 0:V], ps[:, 0:V])
                nc.vector.tensor_add(stage1[:, 0:V], enc_dup[1][:, 0:V], ps[:, 0:V])
                nc.sync.dma_start(out[0:128, u * V:(u + 1) * V], stage0[:, 0:V])
                nc.sync.dma_start(out[128:256, u * V:(u + 1) * V], stage1[:, 0:V])

    nc.compile()
    return nc


def _get_compiled():
    global _COMPILED
    if _COMPILED is None:
        _COMPILED = _build()
    return _COMPILED


def _in_maps(encoder_out, predictor_out, W, b):
    sel = np.zeros((128, NE * 128), dtype=np.float32)
    for r in range(NE):
        sel[r, r * 128:(r + 1) * 128] = 1.0      # selects even u = 2r
    for r in range(NO):
        sel[64 + r, r * 128:(r + 1) * 128] = 1.0  # selects odd u = 2r+1
    ones = np.ones((1, 128), dtype=np.float32)
    bias = np.ascontiguousarray(b.reshape(1, V).astype(np.float32))
    Wc = np.ascontiguousarray(W.astype(np.float32))
    eo = list(range(0, U1, 2)) + list(range(1, U1, 2))
    maps = []
    for i in range(B):
        pT = predictor_out[i].T.astype(np.float32)  # [D, U1]
        maps.append({
            "encT": np.ascontiguousarray(encoder_out[i].T.astype(np.float32)),
            "predT": np.ascontiguousarray(pT[:, eo]),
            "W": Wc,
            "bias": bias,
            "ones": ones,
            "sel": sel,
        })
    return maps


def run(encoder_out, predictor_out, W, b, trace=False, tmpdir=None):
    from concourse.bass_utils import run_bass_kernel_spmd

    nc = _get_compiled()
    maps = _in_maps(encoder_out, predictor_out, W, b)
    res = run_bass_kernel_spmd(
        nc, maps, list(range(B)), trace=trace,
        **({"tmpdir": tmpdir} if tmpdir else {}))
    outs = np.stack([res.results[i]["out"].reshape(T, U1, V) for i in range(B)])
    return outs, res


def kernel(encoder_out, predictor_out, W, b):
    outs, _ = run(encoder_out, predictor_out, W, b)
    return outs



# revision 6
# speedup vs baseline: 1.1788x; 1.1788x over previous
"""RNN-T Joiner kernel for Trainium2 (Bass/Tile), 8-core data-parallel over batch.

out[b,t,u,v] = (enc[b,t] @ We)[v] + (pred[b,u] @ Wp)[v] + bias[v]

Per core (one batch element). The 68.2 MB output store is the roofline, so
the pipeline is arranged to start storing as early as possible and never
starve the DMA engines:

  - sel one-hot (for PE row-broadcast) is generated on-device (gpsimd memset
    + affine_select) instead of a 2.2 MB DMA load on the critical path.
  - Input loads are issued in dependency order (predT, Wp, bias, encT, We)
    with per-chunk W DMAs so projections start as chunks land.
  - enc projection chunk loop is c-outer so each We chunk is consumed right
    as its DMA completes.
  - Main loop: PE broadcasts pred rows via one-hot f32r matmuls into PSUM
    (computed once per u-pair, shared by both t-halves). The PSUM drain is
    split across three engines: DVE adds t-half 0 directly from PSUM,
    Scalar copies the pair to SBUF, gpsimd (SBUF-only engine) adds t-half 1
    from that copy. No single engine gates the stores.
  - Stores: 4-u blocks (2 MB per DMA, 16 KB descriptors), stage pools
    bufs=3 for pipelining slack.
"""

import sys

sys.path.insert(0, "/opt/trn_rl_repo")

import numpy as np

B, T, U1, D, V = 8, 256, 65, 640, 1024
KC = D // 128   # 5 contraction chunks
NE = (U1 + 1) // 2  # 33 even u rows (0,2,..,64) at partitions 0..32
NO = U1 // 2        # 32 odd u rows (1,3,..,63) at partitions 64..95
UBLK = 4            # u's per output DMA block: 16 blocks x 4 + tail u=64

_COMPILED = None


def _build():
    import concourse.bacc as bacc
    import concourse.tile as tile
    import concourse.mybir as mybir

    f32 = mybir.dt.float32
    f32r = mybir.dt.float32r

    nc = bacc.Bacc("TRN2", target_bir_lowering=False, debug=False, num_devices=8)

    encT = nc.dram_tensor("encT", [D, T], f32, kind="ExternalInput")
    # predT columns: even u's (0,2,..,64) then odd u's (1,3,..,63)
    predT = nc.dram_tensor("predT", [D, U1], f32, kind="ExternalInput")
    W = nc.dram_tensor("W", [2 * D, V], f32, kind="ExternalInput")
    bias = nc.dram_tensor("bias", [1, V], f32, kind="ExternalInput")
    ones = nc.dram_tensor("ones", [1, 128], f32, kind="ExternalInput")
    out = nc.dram_tensor("out", [T, U1 * V], f32, kind="ExternalOutput")

    with tile.TileContext(nc) as tc:
        with tc.tile_pool(name="consts", bufs=1) as cp:
            # ---- on-device sel generation (gpsimd; overlaps input DMAs) ----
            # sel[r, r*128:(r+1)*128] = 1 for r<NE (even u=2r);
            # sel[64+r, r*128:(r+1)*128] = 1 for r<NO (odd u=2r+1).
            # Rows 33..63 / 96..127 are never read as lhsT.
            sel = cp.tile([128, NE * 128], f32r, tag="sel")
            pred_sp = cp.tile([128, V], f32r, tag="pred_sp")
            enc_dup = []
            for tt in range(2):
                ed = cp.tile([128, 2 * V], f32, tag=f"enc_dup{tt}")
                enc_dup.append(ed)

            with tc.tile_pool(name="wpool", bufs=1) as wp:
                # sel built in f32 scratch (gpsimd can't write f32r), then
                # rounded into the f32r tile via DVE tensor_copy.
                self_ = wp.tile([128, NE * 128], f32, tag="self_")
                nc.gpsimd.memset(self_[:], 1.0)
                nc.gpsimd.affine_select(
                    self_[0:64, :].rearrange("p (s j) -> p s j", s=NE),
                    self_[0:64, :].rearrange("p (s j) -> p s j", s=NE),
                    pattern=[[1, NE], [0, 128]],
                    compare_op=mybir.AluOpType.is_equal,
                    fill=0.0, base=0, channel_multiplier=-1)
                nc.gpsimd.affine_select(
                    self_[64:128, :].rearrange("p (s j) -> p s j", s=NE),
                    self_[64:128, :].rearrange("p (s j) -> p s j", s=NE),
                    pattern=[[1, NE], [0, 128]],
                    compare_op=mybir.AluOpType.is_equal,
                    fill=0.0, base=0, channel_multiplier=-1)
                nc.vector.tensor_copy(sel[:], self_[:])

                # ---- input loads, dependency order ----
                predT_sb = wp.tile([128, KC * U1], f32, tag="predT")
                nc.sync.dma_start(
                    predT_sb[:].rearrange("p (c u) -> p c u", c=KC),
                    predT[:].rearrange("(c p) u -> p c u", p=128))
                Wp_sb = []
                for c in range(KC):
                    t_ = wp.tile([128, V], f32, tag=f"Wp{c}")
                    nc.sync.dma_start(t_[:], W[D + c * 128:D + (c + 1) * 128, :])
                    Wp_sb.append(t_)
                bias_sb = wp.tile([1, V], f32, tag="bias")
                nc.sync.dma_start(bias_sb[:], bias[:])
                ones_sb = wp.tile([1, 128], f32, tag="ones")
                nc.sync.dma_start(ones_sb[:], ones[:])
                encT_sb = wp.tile([128, KC * T], f32, tag="encT")
                nc.sync.dma_start(
                    encT_sb[:].rearrange("p (c t) -> p c t", c=KC),
                    encT[:].rearrange("(c p) t -> p c t", p=128))
                We_sb = []
                for c in range(KC):
                    t_ = wp.tile([128, V], f32, tag=f"We{c}")
                    nc.sync.dma_start(t_[:], W[c * 128:(c + 1) * 128, :])
                    We_sb.append(t_)

                # ---- projections (fp32 matmuls) ----
                with tc.tile_pool(name="spsum", bufs=2, space="PSUM") as sp:
                    ps_p = sp.tile([128, V], f32, tag="ps")
                    for vt in range(2):
                        vs = slice(vt * 512, (vt + 1) * 512)
                        for c in range(KC):
                            nc.tensor.matmul(
                                ps_p[0:NE, vs],
                                predT_sb[:, c * U1:c * U1 + NE],
                                Wp_sb[c][:, vs],
                                start=(c == 0), stop=False)
                        nc.tensor.matmul(
                            ps_p[0:NE, vs], ones_sb[0:1, 0:NE], bias_sb[0:1, vs],
                            start=False, stop=True)
                    for vt in range(2):
                        vs = slice(vt * 512, (vt + 1) * 512)
                        for c in range(KC):
                            nc.tensor.matmul(
                                ps_p[64:64 + NO, vs],
                                predT_sb[:, c * U1 + NE:(c + 1) * U1],
                                Wp_sb[c][:, vs],
                                start=(c == 0), stop=False)
                        nc.tensor.matmul(
                            ps_p[64:64 + NO, vs], ones_sb[0:1, 0:NO], bias_sb[0:1, vs],
                            start=False, stop=True)
                    nc.vector.tensor_copy(pred_sp[0:NE, :], ps_p[0:NE, :])
                    nc.vector.tensor_copy(pred_sp[64:64 + NO, :], ps_p[64:64 + NO, :])

                    # enc: c-outer so chunk c is consumed as its DMA lands
                    for tt in range(2):
                        ps_e = sp.tile([128, V], f32, tag="pse")
                        for c in range(KC):
                            for vt in range(2):
                                vs = slice(vt * 512, (vt + 1) * 512)
                                nc.tensor.matmul(
                                    ps_e[:, vs],
                                    encT_sb[:, c * T + tt * 128:c * T + (tt + 1) * 128],
                                    We_sb[c][:, vs],
                                    start=(c == 0), stop=(c == KC - 1))
                        nc.vector.tensor_copy(enc_dup[tt][:, 0:V], ps_e[:])
                        nc.scalar.copy(enc_dup[tt][:, V:2 * V], ps_e[:])

            def bcast_mm(ps_ap, u, vt):
                # one [128,512] slice of pred_b[u] broadcast to all partitions
                vs = slice(vt * 512, (vt + 1) * 512)
                if u % 2 == 0:
                    nc.tensor.matmul(
                        ps_ap,
                        sel[0:NE, (u // 2) * 128:(u // 2 + 1) * 128],
                        pred_sp[0:NE, vs], start=True, stop=True)
                else:
                    nc.tensor.matmul(
                        ps_ap,
                        sel[64:64 + NO, (u // 2) * 128:(u // 2 + 1) * 128],
                        pred_sp[64:64 + NO, vs], start=True, stop=True)

            def do_pair(ps_ap, pred_sb_ap, stage0_ap, stage1_ap, nv):
                # DVE adds t-half0 from PSUM; Scalar copies pair to SBUF;
                # gpsimd adds t-half1 from the SBUF copy.
                nc.vector.tensor_tensor(
                    stage0_ap, enc_dup[0][:, 0:nv], ps_ap,
                    mybir.AluOpType.add)
                nc.scalar.copy(pred_sb_ap, ps_ap)
                nc.gpsimd.tensor_tensor(
                    stage1_ap, enc_dup[1][:, 0:nv], pred_sb_ap,
                    mybir.AluOpType.add)

            # ---- main loop: broadcast + add + store ----
            with tc.tile_pool(name="outp", bufs=3) as op_, \
                 tc.tile_pool(name="pairp", bufs=3) as pp2, \
                 tc.tile_pool(name="mpsum", bufs=2, space="PSUM") as mp:
                for blk in range(16):
                    u0 = blk * UBLK
                    stage0 = op_.tile([128, UBLK * V], f32, tag="stage0")
                    stage1 = op_.tile([128, UBLK * V], f32, tag="stage1")
                    for pair in range(UBLK // 2):
                        ua = u0 + 2 * pair
                        ps = mp.tile([128, 2048], f32, tag="mps")
                        pred_sb = pp2.tile([128, 2048], f32, tag="pred_sb")
                        bcast_mm(ps[:, 0:512], ua, 0)
                        bcast_mm(ps[:, 1024:1536], ua + 1, 0)
                        bcast_mm(ps[:, 512:1024], ua, 1)
                        bcast_mm(ps[:, 1536:2048], ua + 1, 1)
                        do_pair(
                            ps[:], pred_sb[:],
                            stage0[:, pair * 2048:(pair + 1) * 2048],
                            stage1[:, pair * 2048:(pair + 1) * 2048], 2048)
                    nc.sync.dma_start(
                        out[0:128, u0 * V:(u0 + UBLK) * V], stage0[:])
                    nc.sync.dma_start(
                        out[128:256, u0 * V:(u0 + UBLK) * V], stage1[:])
                # tail u = 64
                u = U1 - 1
                stage0 = op_.tile([128, UBLK * V], f32, tag="stage0")
                stage1 = op_.tile([128, UBLK * V], f32, tag="stage1")
                ps = mp.tile([128, 2048], f32, tag="mps")
                pred_sb = pp2.tile([128, 2048], f32, tag="pred_sb")
                bcast_mm(ps[:, 0:512], u, 0)
                bcast_mm(ps[:, 512:1024], u, 1)
                do_pair(ps[:, 0:V], pred_sb[:, 0:V],
                        stage0[:, 0:V], stage1[:, 0:V], V)
                nc.sync.dma_start(out[0:128, u * V:(u + 1) * V], stage0[:, 0:V])
                nc.sync.dma_start(out[128:256, u * V:(u + 1) * V], stage1[:, 0:V])

    nc.compile()
    return nc


def _get_compiled():
    global _COMPILED
    if _COMPILED is None:
        _COMPILED = _build()
    return _COMPILED


def _in_maps(encoder_out, predictor_out, W, b):
    ones = np.ones((1, 128), dtype=np.float32)
    bias = np.ascontiguousarray(b.reshape(1, V).astype(np.float32))
    Wc = np.ascontiguousarray(W.astype(np.float32))
    eo = list(range(0, U1, 2)) + list(range(1, U1, 2))
    maps = []
    for i in range(B):
        pT = predictor_out[i].T.astype(np.float32)  # [D, U1]
        maps.append({
            "encT": np.ascontiguousarray(encoder_out[i].T.astype(np.float32)),
            "predT": np.ascontiguousarray(pT[:, eo]),
            "W": Wc,
            "bias": bias,
            "ones": ones,
        })
    return maps


def run(encoder_out, predictor_out, W, b, trace=False, tmpdir=None):
    from concourse.bass_utils import run_bass_kernel_spmd

    nc = _get_compiled()
    maps = _in_maps(encoder_out, predictor_out, W, b)
    res = run_bass_kernel_spmd(
        nc, maps, list(range(B)), trace=trace,
        **({"tmpdir": tmpdir} if tmpdir else {}))
    outs = np.stack([res.results[i]["out"].reshape(T, U1, V) for i in range(B)])
    return outs, res


def kernel(encoder_out, predictor_out, W, b):
    outs, _ = run(encoder_out, predictor_out, W, b)
    return outs
